# revision 37
# baseline (speedup 1.0000x reference)
"""Trainium2 Bass kernel for causal MultiHeadAttention.

Problem: B=4, S=2048, H=16, D=64, DM=1024, fp32 I/O.
  qkv = x @ w_qkv ; causal softmax attention per head ; out = attn @ w_out

Sharding (8 cores): 4-way batch x 2-way heads. Core c handles batch c//2 and
heads (c%2)*8 .. +8. Each core computes a partial out-projection (its 512
attention channels x full w_out row-slice); the host sums the two head-half
partials per batch while unsharding.

Per-core dataflow (bf16 matmul inputs, fp32 PSUM):
  xt = x[b].T (host)                                  [1024, 2048]
  qT,kT = w.T-major proj:  lhsT=w tiles, rhs=xt        -> [512ch, 2048]
  v    = row-major proj:   lhsT=xt tiles, rhs=w_v      -> v_rm slots
  scoresT[ki,qi] = kT.T @ qT  (K=64, two heads row-split in the PE array)
  probsT = exp(scoresT + causal_addmask)  (no max pass: |scores| <= ~6)
  PV per head-pair block:
    even head lhsT = [v | ones]            (M=65: vals @ parts 0-63, denom @ 64)
    odd  head lhsT = [ones | 0*63 | v]     (M=128: denom @ part 0, vals @ 64-127)
  so the odd head's output lands at partitions 64-127 directly (no shift DMA).
  normalize: recip rows read straight from PSUM; two K=1 broadcast matmuls at
  tile_position (64,0) and (0,64) run concurrently; one [128,512] DVE multiply
  writes both heads of oT.
  partial_out = oT.T @ w_out_slice (row-major psum -> sbuf -> HBM)

The attention inner loop is software-pipelined one block ahead (scores(i+1)
is emitted before PV(i)) so the exp on the Act engine overlaps PE work, and
projection/out-proj work is drip-fed as single-matmul filler units sized by a
per-block slack budget. Fillers carry deadlines (v tiles before the PV that
reads them; pair p's q/k before pair p starts); out-proj row-tiles of the
last pair are scheduled as fillers as soon as their query supertile is
normalized.
"""

from collections import deque

import numpy as np

B, S, H, D = 4, 2048, 16, 64
DM = H * D          # 1024
NCORES = 8
HPC = H // 2        # 8 heads per core
CQ = HPC * D        # 512 channels per core
NEG = -1.0e9

SLOT = 130          # v_rm per-pair slot: [v|1]=65 even + [v|1]=65 odd
_PROG_CACHE = {}


def build_program(rep_qkv=1, rep_attn=1, rep_oproj=1, rep_all=1, tune=None):
    tune = dict(tune or {})
    PRBUFS = tune.get("pr_bufs", 8)
    OUT_BF16 = tune.get("out_bf16", True)
    DEFER_KB = tune.get("defer_kb", 1)
    JIT_QK = tune.get("jit_qk", True)
    PV_LAG = tune.get("pv_lag", 2)
    NO_ATTN = tune.get("no_attn", False)    # ablation: GEMM stream only
    ATTN_ONLY = tune.get("attn_only", False)  # ablation: attention loop only
    if NO_ATTN or ATTN_ONLY:
        JIT_QK = False
    import concourse.mybir as mybir
    import concourse.tile as tile
    from concourse import bacc

    dt = mybir.dt
    f32 = dt.float32
    bf16 = dt.bfloat16
    AF = mybir.ActivationFunctionType

    nc = bacc.Bacc(None)
    xt = nc.declare_dram_parameter("xt", [DM, S], bf16, isOutput=False)
    wqk = nc.declare_dram_parameter("wqk", [DM, 2 * CQ], bf16, isOutput=False)
    wv = nc.declare_dram_parameter("wv", [DM, CQ], bf16, isOutput=False)
    wo = nc.declare_dram_parameter("wo", [CQ, DM], bf16, isOutput=False)
    mask = nc.declare_dram_parameter("mask", [128, 256], bf16, isOutput=False)
    out = nc.declare_dram_parameter("out", [S, DM], bf16 if OUT_BF16 else f32,
                                    isOutput=True)

    KT = DM // 128      # 8 contraction tiles over model dim
    NRT = S // 128      # 16 row tiles over sequence
    NRC = S // 512      # 4 row chunks over sequence
    NP = HPC // 2       # 4 head pairs per core
    NST = S // 512      # 4 query supertiles

    with tile.TileContext(nc) as tc:
        with (
            tc.tile_pool(name="persist", bufs=1) as pp,
            tc.tile_pool(name="probs", bufs=PRBUFS) as probsp,
            tc.tile_pool(name="norm", bufs=4) as normp,
            tc.tile_pool(name="ostage", bufs=3) as ostagep,
            tc.tile_pool(name="psmm", bufs=2, space="PSUM") as psmm,
            tc.tile_pool(name="pssc", bufs=2, space="PSUM") as pssc,
            tc.tile_pool(name="psout", bufs=2, space="PSUM") as psout,
        ):
            # ---- load inputs to SBUF ----
            xt_sb = []
            wqk_sb = []
            wv_sb = []
            # xt + wqk gate the first qk filler group: stripe them over four
            # DMA queues so that set completes earliest; wv next, wo last
            qs = [nc.sync, nc.scalar, nc.gpsimd]
            for i in range(KT):
                t = pp.tile([128, S], bf16, tag=f"xt{i}", name=f"xt{i}")
                qs[(2 * i) % 3].dma_start(out=t[:], in_=xt[128 * i:128 * (i + 1), :])
                xt_sb.append(t)
                t = pp.tile([128, 2 * CQ], bf16, tag=f"wqk{i}", name=f"wqk{i}")
                qs[(2 * i + 1) % 3].dma_start(out=t[:], in_=wqk[128 * i:128 * (i + 1), :])
                wqk_sb.append(t)
            for i in range(KT):
                t = pp.tile([128, CQ], bf16, tag=f"wv{i}", name=f"wv{i}")
                qs[i % 3].dma_start(out=t[:], in_=wv[128 * i:128 * (i + 1), :])
                wv_sb.append(t)
            wo_sb = []
            for c in range(CQ // 128):
                t = pp.tile([128, DM], bf16, tag=f"wo{c}", name=f"wo{c}")
                qs[c % 3].dma_start(out=t[:], in_=wo[128 * c:128 * (c + 1), :])
                wo_sb.append(t)
            # [tri01 | identity]: tri01 for post-exp causal zeroing,
            # bf16 identity as the PE-transpose rhs
            mask_sb = pp.tile([128, 256], bf16, tag="mask", name="mask")
            nc.sync.dma_start(out=mask_sb[:], in_=mask[:, :])
            tri01 = mask_sb[:, 0:128]
            id_sb = mask_sb[:, 128:256]

            # persistent activation tensors
            qT = [pp.tile([128, S], bf16, tag=f"qT{p}", name=f"qT{p}") for p in range(NP)]
            kT = [pp.tile([128, S], bf16, tag=f"kT{p}", name=f"kT{p}") for p in range(NP)]
            v_rm = [pp.tile([128, NP * SLOT], bf16, tag=f"v{rt}", name=f"v{rt}") for rt in range(NRT)]
            oT = [pp.tile([128, S], bf16, tag=f"oT{p}", name=f"oT{p}") for p in range(NP)]

            if NO_ATTN:
                for p in range(NP):
                    nc.vector.memset(oT[p][:], 0.0)
            if ATTN_ONLY:
                for p in range(NP):
                    nc.vector.memset(qT[p][:], 0.01)
                    nc.vector.memset(kT[p][:], 0.01)
                for rt in range(NRT):
                    vw0 = v_rm[rt].rearrange("p (g s) -> p g s", s=SLOT)
                    nc.vector.memset(vw0[:, :, 0:64], 0.01)
                    nc.vector.memset(vw0[:, :, 65:129], 0.01)
            # static v_rm ones columns (denominator trick): col 64 even,
            # col 129 odd
            for rt in range(NRT):
                vw = v_rm[rt].rearrange("p (g s) -> p g s", s=SLOT)
                nc.vector.memset(vw[:, :, 64:65], 1.0)
                nc.vector.memset(vw[:, :, 129:130], 1.0)

            # ---------- filler unit machinery ----------
            # Units: [cost_ns, fn, emitted]. A tag maps to the (start, end)
            # queue range providing it; need(tag) emits just that range, out
            # of order if necessary. emit_units walks forward skipping
            # already-emitted entries.
            queue = []
            provided = {}    # tag -> (start, end) unit range
            ptr = {"pos": 0}

            def add_unit(cost, fn, head=False):
                queue.append([cost, fn, False, head])

            def mark(tag, start):
                provided[tag] = (start, len(queue))

            def emit_range(a, b):
                for i in range(a, b):
                    e = queue[i]
                    if not e[2]:
                        e[1]()
                        e[2] = True

            def close_open_group():
                # finish any half-emitted accumulation group at the pointer
                # so out-of-order pulls never open a second PSUM group
                pos = ptr["pos"]
                while pos < len(queue):
                    e = queue[pos]
                    if e[2]:
                        pos += 1
                        continue
                    if e[3]:
                        break  # next unit starts a fresh group: closed
                    e[1]()
                    e[2] = True
                    pos += 1
                ptr["pos"] = pos

            def need(tag):
                if tag in provided:
                    a, b = provided[tag]
                    if not (a <= ptr["pos"] and all(e[2] for e in queue[a:b])):
                        close_open_group()
                    emit_range(a, b)

            def emit_units(budget=None):
                # budget breaks only at group heads so a PSUM accumulation
                # group is never left half-emitted (pool-cycle hazard)
                pos = ptr["pos"]
                spent = 0
                while pos < len(queue):
                    e = queue[pos]
                    if e[2]:
                        pos += 1
                        continue
                    if budget is not None and spent + e[0] > budget:
                        break
                    e[1]()
                    e[2] = True
                    spent += e[0]
                    pos += 1
                ptr["pos"] = pos
                return spent

            # ---------- projection group builders (emit as units) ----------
            def qk_group(p, ct, rc):
                if ATTN_ONLY:
                    return
                dst = qT[p] if ct < NP else kT[p]
                g0 = len(queue)
                ps_box = {}
                for kt in range(KT):
                    def mm(kt=kt, ct=ct, rc=rc, ps_box=ps_box):
                        if kt == 0:
                            ps_box["ps"] = psmm.tile([128, 512], f32, tag="mm", name="mm")
                        nc.tensor.matmul(
                            ps_box["ps"][:],
                            lhsT=wqk_sb[kt][:, 128 * ct:128 * (ct + 1)],
                            rhs=xt_sb[kt][:, 512 * rc:512 * (rc + 1)],
                            start=(kt == 0),
                            stop=(kt == KT - 1),
                        )
                    add_unit(230, mm, head=(kt == 0))
                def evict(dst=dst, rc=rc, ps_box=ps_box):
                    nc.vector.tensor_copy(
                        dst[:, 512 * rc:512 * (rc + 1)], ps_box["ps"][:]
                    )
                add_unit(60, evict)
                mark(("qkg", p, ct, rc), g0)

            def qk_units(p):
                # Q then K column-tiles of pair p, all row chunks
                g0 = len(queue)
                for ct in (p, NP + p):
                    for rc in range(NRC):
                        qk_group(p, ct, rc)
                mark(("qk", p), g0)

            def v_units(rt):
                if ATTN_ONLY:
                    return
                g0 = len(queue)
                ps_box = {}
                for kt in range(KT):
                    def mm(kt=kt, rt=rt, ps_box=ps_box):
                        if kt == 0:
                            ps_box["ps"] = psmm.tile([128, 512], f32, tag="mm", name="mm")
                        nc.tensor.matmul(
                            ps_box["ps"][:],
                            lhsT=xt_sb[kt][:, 128 * rt:128 * (rt + 1)],
                            rhs=wv_sb[kt][:],
                            start=(kt == 0),
                            stop=(kt == KT - 1),
                        )
                    add_unit(230, mm, head=(kt == 0))
                def evict(rt=rt, ps_box=ps_box):
                    ps = ps_box["ps"]
                    psv = ps.rearrange("p (h c) -> p h c", c=64)
                    vw = v_rm[rt].rearrange("p (g s) -> p g s", s=SLOT)
                    nc.vector.tensor_copy(vw[:, :, 0:64], psv[:, 0::2, :])
                    nc.vector.tensor_copy(vw[:, :, 65:129], psv[:, 1::2, :])
                add_unit(120, evict)
                mark(("v", rt), g0)

            def oproj_units(rt):
                if ATTN_ONLY:
                    return
                st_box = {}
                for o2 in range(2):
                    ps_box = {}
                    for c in range(4):
                        def mm(c=c, o2=o2, rt=rt, ps_box=ps_box, st_box=st_box):
                            if c == 0 and o2 == 0:
                                st_box["sb"] = ostagep.tile(
                                    [128, 1024], bf16 if OUT_BF16 else f32,
                                    tag="ostage", name="ostage"
                                )
                            if c == 0:
                                ps_box["ps"] = psmm.tile([128, 512], f32, tag="mm", name="mm")
                            nc.tensor.matmul(
                                ps_box["ps"][:],
                                lhsT=oT[c][:, 128 * rt:128 * (rt + 1)],
                                rhs=wo_sb[c][:, 512 * o2:512 * (o2 + 1)],
                                start=(c == 0),
                                stop=(c == 3),
                            )
                        add_unit(230, mm, head=(c == 0))
                    def evict(o2=o2, ps_box=ps_box, st_box=st_box):
                        nc.vector.tensor_copy(
                            st_box["sb"][:, 512 * o2:512 * (o2 + 1)], ps_box["ps"][:]
                        )
                    add_unit(60, evict)
                def dma(rt=rt, st_box=st_box):
                    eng = nc.sync if rt % 2 == 0 else nc.gpsimd
                    eng.dma_start(
                        out=out[128 * rt:128 * (rt + 1), :], in_=st_box["sb"][:]
                    )
                add_unit(30, dma)

            # ---------- attention ----------
            def emit_sc_exp(p, st, kb):
                """Scores + mask + exp for one block; returns (pr, qi0)."""
                r = kb - 4 * st
                qi0 = 128 * r if r > 0 else 0
                sc = pssc.tile([128, 1024], f32, tag="sc", name="sc")
                for hh in range(2):
                    base, lo = 512 * hh, 64 * hh
                    nc.tensor.matmul(
                        sc[:, base + qi0:base + 512],
                        lhsT=kT[p][lo:lo + 64, 128 * kb:128 * (kb + 1)],
                        rhs=qT[p][lo:lo + 64, 512 * st + qi0:512 * (st + 1)],
                        start=True,
                        stop=True,
                        tile_position=(lo, 0),
                    )
                pr = probsp.tile([128, 1024], bf16, tag="pr", name="pr")
                if qi0 == 0:
                    nc.scalar.activation(pr[:], sc[:], AF.Exp)
                else:
                    pr_v = pr.rearrange("p (h q) -> p h q", h=2)
                    sc_v = sc.rearrange("p (h q) -> p h q", h=2)
                    nc.scalar.activation(
                        pr_v[:, :, qi0:512], sc_v[:, :, qi0:512], AF.Exp
                    )
                if r >= 0:
                    # causal zeroing after exp (keeps DVE off the Act path):
                    # strip cols [qi0, qi0+128) follow the same strictly-
                    # lower-tri pattern (p > jj) for every diagonal block
                    for hh in range(2):
                        base = 512 * hh
                        nc.gpsimd.tensor_mul(
                            pr[:, base + qi0:base + qi0 + 128],
                            pr[:, base + qi0:base + qi0 + 128],
                            tri01,
                        )
                return pr, qi0

            def emit_pv(p, out_ps, st, kb, pr):
                """q-major PV: chunk j (128 q) x head h -> [128q, 65] (vals+den).

                out_ps = (tileA, tileB), each [128, 260] holding two chunks'
                slots of 65 cols. Chunk j accumulates kb = 0 .. 4*st+j.
                """
                need(("v", kb))
                r = kb - 4 * st
                vw = v_rm[kb].rearrange("p (g s) -> p g s", s=SLOT)
                # PSUM accumulation groups are BANK-granular (start zeroes the
                # whole 2KB zero-region): exactly one start (first matmul at
                # kb==0) and one stop (the bank's last matmul: odd chunk's
                # diagonal block, odd head) per tile per supertile.
                for j in range(max(r, 0), 4):
                    t = out_ps[j // 2]
                    for h in range(2):
                        col0 = 130 * (j % 2) + 65 * h
                        nc.tensor.matmul(
                            t[:, col0:col0 + 65],
                            lhsT=pr[:, 512 * h + 128 * j:512 * h + 128 * (j + 1)],
                            rhs=vw[:, p, 65 * h:65 * h + 65],
                            start=(kb == 0 and h == 0 and j % 2 == 0),
                            stop=(j % 2 == 1 and kb == 4 * st + j and h == 1),
                        )

            def normalize_half(p, st, out_ps, half, box):
                """recip + per-partition scale of one half-supertile (2 chunks).

                box carries the shared bf16 stage tile across halves; the
                second half returns the deferred post (PE transposes + oT
                eviction)."""
                t = out_ps[half]
                v4 = t.rearrange("p (s c) -> p s c", c=65)
                rc = normp.tile([128, 4], f32, tag="rc", name="rc")
                nc.vector.reciprocal(rc[:, 0:4], v4[:, :, 64])
                if half == 0:
                    box["stage"] = normp.tile([128, 512], bf16, tag="stage",
                                              name="stage")
                stage = box["stage"]
                for jl in range(2):
                    j = 2 * half + jl
                    for h in range(2):
                        s = 2 * jl + h
                        nc.vector.tensor_scalar_mul(
                            stage[:, 128 * j + 64 * h:128 * j + 64 * h + 64],
                            v4[:, s, 0:64],
                            rc[:, s:s + 1],
                        )
                if half == 0:
                    return None

                def post():
                    tp_ps = psmm.tile([128, 512], f32, tag="mm", name="mm")
                    tpb = tp_ps[:].bitcast(bf16)
                    # one bank-wide group: start zeroes the bank, chunks land
                    # in disjoint col ranges, single stop; bf16 transposes
                    # (1 cyc/row) into a bitcast view of the f32 mm tile
                    for j in range(4):
                        nc.tensor.matmul(
                            tpb[:, 128 * j:128 * (j + 1)],
                            lhsT=stage[:, 128 * j:128 * (j + 1)],
                            rhs=id_sb[:],
                            is_transpose=True,
                            start=(j == 0),
                            stop=(j == 3),
                        )
                    nc.vector.tensor_copy(
                        oT[p][:, 512 * st:512 * (st + 1)], tpb[:, 0:512]
                    )

                return post

            # ---------- schedule ----------
            ACT_OVH = tune.get("act_ovh", 295)   # per-activation fixed ns
            ACT_RATE = tune.get("act_rate", 1.2)  # exp cols per ns
            BLK_PAD = tune.get("blk_pad", 0)      # extra per-block HW latency

            for _arep in range(rep_all):
             pending_post = []
             provided[("repstart", _arep)] = (0, len(queue))
             first_flush = [True]

             if JIT_QK:
                 if _arep == 0:
                     for rc in range(NRC):
                         qk_group(0, 0, rc)
                         qk_group(0, NP, rc)
                         for rt in range(4 * rc, 4 * rc + 4):
                             v_units(rt)
                 else:
                     # qk(0) was queued during the previous rep's last pair
                     for rt in range(NRT):
                         v_units(rt)
                 for _qrep in range(max(rep_qkv, 1) - 1):
                     qk_units(0)
                 for p in range(1, NP):
                     for _qrep in range(max(rep_qkv, 1)):
                         qk_units(p)
                 # pre-phase: only q/k row-chunk 0 of pair 0 + v tile 0
                 need(("qkg", 0, 0, 0))
                 need(("qkg", 0, NP, 0))
                 need(("v", 0))
             else:
                 for _qrep in range(max(rep_qkv, 1)):
                     qk_units(0)
                 for rt in range(NRT):
                     v_units(rt)
                 for p in range(1, NP):
                     for _qrep in range(max(rep_qkv, 1)):
                         qk_units(p)
                 # pre-phase: q/k of pair 0 + first v tiles
                 need(("qk", 0))
                 need(("v", 3))

             if NO_ATTN:
                 emit_units()
                 for rt in range(NRT):
                     oproj_units(rt)
                 emit_units()
                 continue

             for p in range(NP):
                 if JIT_QK:
                     need(("qkg", p, p, 0))
                     need(("qkg", p, NP + p, 0))
                     if p == NP - 1 and _arep + 1 < rep_all:
                         # queue next rep's q/k of pair 0 as fillers for the
                         # last pair's slack (qT[0]/kT[0] are long dead here)
                         for rc in range(NRC):
                             qk_group(0, 0, rc)
                             qk_group(0, NP, rc)
                 else:
                     need(("qk", p))
                 def finish_st(prev):
                     # prev = (out_ps, st, kb, pr, box)
                     out_ps, pst, pkb, ppr, pbox = prev
                     emit_pv(p, out_ps, pst, pkb, ppr)
                     post = normalize_half(p, pst, out_ps, 1, pbox)
                     if p == NP - 1 and rep_attn == 1:
                         def put(pst=pst, post=post):
                             post()
                             for rt in range(4 * pst, 4 * pst + 4):
                                 oproj_units(rt)
                         pending_post.append(put)
                     else:
                         pending_post.append(post)

                 def process(ent):
                     if ent[2] == 4 * ent[1] + 3:
                         finish_st(ent)    # last block of its supertile
                     else:
                         emit_pv(p, ent[0], ent[1], ent[2], ent[3])
                         if ent[2] == 4 * ent[1] + 1:
                             # first half-supertile complete: free tileA
                             normalize_half(p, ent[1], ent[0], 0, ent[4])

                 blocks = [
                     (st, kb)
                     for st in range(NST)
                     for _ in range(rep_attn)
                     for kb in range(4 * st + 4)
                 ]
                 out_ps = None
                 norm_box = None
                 pend = deque()   # PV lags exp by PV_LAG blocks: accumulation
                 deficit = 0      # is commutative, and an aged pr is sem-ready
                 blk = 0          # so the in-order PE never stalls on Act
                 for st, kb in blocks:
                     if kb == 0:
                         if JIT_QK:
                             need(("qkg", p, p, st))
                         out_ps = [
                             psout.tile([128, 260], f32, tag="o", name="o")
                             for _ in range(2)
                         ]
                         norm_box = {}
                     if JIT_QK:
                         need(("qkg", p, NP + p, kb // 4))
                     pr, qi0 = emit_sc_exp(p, st, kb)
                     ncols = 2 * (512 - qi0)
                     act_ns = ncols / ACT_RATE + ACT_OVH + BLK_PAD
                     r_ = kb - 4 * st
                     # sc matmuls + q-major PV (2 heads x (4-max(r,0)) chunks
                     # of 65 cols) + per-block fixed overhead
                     pe_ns = ncols / 2.4 + 130 * (4 - max(r_, 0)) / 2.4 + 60
                     deficit += act_ns - pe_ns
                     if pending_post and blk >= DEFER_KB:
                         if first_flush[0]:
                             # all leftover units from the previous rep
                             # must precede this rep's first oT write
                             need(("repstart", _arep))
                             first_flush[0] = False
                         for fn in pending_post:
                             fn()
                         pending_post.clear()
                         deficit -= 450
                     if deficit > 0:
                         deficit -= emit_units(budget=deficit)
                     while len(pend) >= max(PV_LAG, 1):
                         process(pend.popleft())
                     pend.append((out_ps, st, kb, pr, norm_box))
                     blk += 1
                 while pend:
                     process(pend.popleft())

             for fn in pending_post:
                 fn()
             pending_post.clear()
             if p == NP - 1 and rep_attn != 1:
                 for rt in range(NRT):
                     oproj_units(rt)
             for _orep in range(rep_oproj - 1):
                 for rt in range(NRT):
                     oproj_units(rt)
             if _arep + 1 >= rep_all:
                 emit_units()  # final drain (tail out-proj)
    nc.finalize()
    return nc


def get_program():
    if "nc" not in _PROG_CACHE:
        _PROG_CACHE["nc"] = build_program()
    return _PROG_CACHE["nc"]


def make_in_maps(x, w_qkv, w_out):
    import ml_dtypes

    bf = ml_dtypes.bfloat16
    x = np.asarray(x, dtype=np.float32)
    w_qkv = np.asarray(w_qkv, dtype=np.float32)
    w_out = np.asarray(w_out, dtype=np.float32)
    scale = float(D) ** -0.5
    # post-exp causal zeroing: strictly-lower-tri 0/1 (p > jj masked); the
    # same [128,128] pattern applies to every diagonal block's strip.
    # Identity appended as the PE-transpose rhs.
    p_idx = np.arange(128)[:, None]
    j_idx = np.arange(128)[None, :]
    tri = np.where(p_idx > j_idx, 0.0, 1.0)
    mask = np.concatenate([tri, np.eye(128)], axis=1).astype(bf)
    in_maps = []
    for c in range(NCORES):
        b, hh = c // 2, c % 2
        q0 = CQ * hh
        wq = (w_qkv[:, q0:q0 + CQ] * scale).astype(bf)
        wk = w_qkv[:, DM + q0:DM + q0 + CQ].astype(bf)
        in_maps.append(
            {
                "xt": np.ascontiguousarray(x[b].T).astype(bf),
                "wqk": np.concatenate([wq, wk], axis=1),
                "wv": w_qkv[:, 2 * DM + q0:2 * DM + q0 + CQ].astype(bf),
                "wo": w_out[q0:q0 + CQ, :].astype(bf),
                "mask": mask,
            }
        )
    return in_maps


def gather(results):
    outs = [np.asarray(results[c]["out"], dtype=np.float32) for c in range(NCORES)]
    return np.stack([outs[2 * b] + outs[2 * b + 1] for b in range(B)], axis=0)


def kernel(x, w_qkv, w_out):
    from concourse.bass_utils import run_bass_kernel_spmd

    nc = get_program()
    in_maps = make_in_maps(x, w_qkv, w_out)
    res = run_bass_kernel_spmd(nc, in_maps, list(range(NCORES)))
    return gather(res.results)



# revision 38
# speedup vs baseline: 1.1106x; 1.1106x over previous
"""Trainium2 Bass kernel for causal MultiHeadAttention.

Problem: B=4, S=2048, H=16, D=64, DM=1024, fp32 I/O.
  qkv = x @ w_qkv ; causal softmax attention per head ; out = attn @ w_out

Sharding (8 cores): 4-way batch x 2-way heads. Core c handles batch c//2 and
heads (c%2)*8 .. +8. Each core computes a partial out-projection (its 512
attention channels x full w_out row-slice); the host sums the two head-half
partials per batch while unsharding.

Per-core dataflow (bf16 matmul inputs, fp32 PSUM):
  xt = x[b].T (host)                                  [1024, 2048]
  qT,kT = w.T-major proj:  lhsT=w tiles, rhs=xt        -> [512ch, 2048]
  v    = row-major proj:   lhsT=xt tiles, rhs=w_v      -> v_rm [v|1] slots
  scoresT[ki,qi] = kT.T @ qT  (K=64, two heads row-split in the PE array)
  probsT = exp(scoresT); causal masking AFTER exp (keeps DVE/Pool off the
  Act critical path): the diagonal 128x128 strip is multiplied by a 0/1
  strictly-lower-tri tile on the Pool engine (same pattern every block).
  PV is q-major: for each 128-q chunk j and head h,
    out[128q, 65] += lhsT=pr[:, head h chunk j] @ rhs=[v|1]   (N=65)
  so PV costs 65 cols/chunk instead of 512/head, and the softmax
  denominator rides along as column 64. Chunk j's accumulation stops at
  diagonal block r==j; chunks {0,1}/{2,3} live in two [128,260] PSUM
  tiles (one bank each; one start/stop per bank per supertile - PSUM
  zeroing is bank-granular).
  normalize: DVE recip of the denom cols + 8 per-partition tensor_scalar
  muls -> bf16 stage; 4 bf16 PE transposes (identity rhs, bitcast PSUM
  view) -> oT.
  partial_out = oT.T @ w_out_slice (row-major psum -> sbuf -> HBM)

The attention inner loop is software-pipelined: scores/exp run ahead while
PV lags by pv_lag=2 blocks (PSUM accumulation commutes across blocks, and
an aged pr is semaphore-ready, so the in-order PE never stalls on the Act
engine's exp latency - the main real-HW stall source). Projection/out-proj
work is drip-fed as single-matmul filler units sized by a per-block slack
budget. Fillers carry deadlines (v tiles before the PV that reads them;
pair p's q/k before pair p starts); out-proj row-tiles of the last pair
are scheduled as fillers as soon as their query supertile is normalized.
Input DMAs stripe over all three DMA-capable queues (SP/Act/Pool), with
the first-needed set (xt+wqk) loaded first.
"""

from collections import deque

import numpy as np

B, S, H, D = 4, 2048, 16, 64
DM = H * D          # 1024
NCORES = 8
HPC = H // 2        # 8 heads per core
CQ = HPC * D        # 512 channels per core
NEG = -1.0e9

SLOT = 130          # v_rm per-pair slot: [v|1]=65 even + [v|1]=65 odd
_PROG_CACHE = {}


def build_program(rep_qkv=1, rep_attn=1, rep_oproj=1, rep_all=1, tune=None):
    tune = dict(tune or {})
    PRBUFS = tune.get("pr_bufs", 8)
    OUT_BF16 = tune.get("out_bf16", True)
    DEFER_KB = tune.get("defer_kb", 1)
    JIT_QK = tune.get("jit_qk", True)
    PV_LAG = tune.get("pv_lag", 2)
    NO_ATTN = tune.get("no_attn", False)    # ablation: GEMM stream only
    ATTN_ONLY = tune.get("attn_only", False)  # ablation: attention loop only
    if NO_ATTN or ATTN_ONLY:
        JIT_QK = False
    import concourse.mybir as mybir
    import concourse.tile as tile
    from concourse import bacc

    dt = mybir.dt
    f32 = dt.float32
    bf16 = dt.bfloat16
    AF = mybir.ActivationFunctionType

    nc = bacc.Bacc(None)
    xt = nc.declare_dram_parameter("xt", [DM, S], bf16, isOutput=False)
    wqk = nc.declare_dram_parameter("wqk", [DM, 2 * CQ], bf16, isOutput=False)
    wv = nc.declare_dram_parameter("wv", [DM, CQ], bf16, isOutput=False)
    wo = nc.declare_dram_parameter("wo", [CQ, DM], bf16, isOutput=False)
    mask = nc.declare_dram_parameter("mask", [128, 256], bf16, isOutput=False)
    out = nc.declare_dram_parameter("out", [S, DM], bf16 if OUT_BF16 else f32,
                                    isOutput=True)

    KT = DM // 128      # 8 contraction tiles over model dim
    NRT = S // 128      # 16 row tiles over sequence
    NRC = S // 512      # 4 row chunks over sequence
    NP = HPC // 2       # 4 head pairs per core
    NST = S // 512      # 4 query supertiles

    with tile.TileContext(nc) as tc:
        with (
            tc.tile_pool(name="persist", bufs=1) as pp,
            tc.tile_pool(name="probs", bufs=PRBUFS) as probsp,
            tc.tile_pool(name="norm", bufs=4) as normp,
            tc.tile_pool(name="ostage", bufs=3) as ostagep,
            tc.tile_pool(name="psmm", bufs=2, space="PSUM") as psmm,
            tc.tile_pool(name="pssc", bufs=2, space="PSUM") as pssc,
            tc.tile_pool(name="psout", bufs=2, space="PSUM") as psout,
        ):
            # ---- load inputs to SBUF ----
            xt_sb = []
            wqk_sb = []
            wv_sb = []
            # xt + wqk gate the first qk filler group: stripe them over four
            # DMA queues so that set completes earliest; wv next, wo last
            qs = [nc.sync, nc.scalar, nc.gpsimd]
            for i in range(KT):
                t = pp.tile([128, S], bf16, tag=f"xt{i}", name=f"xt{i}")
                qs[(2 * i) % 3].dma_start(out=t[:], in_=xt[128 * i:128 * (i + 1), :])
                xt_sb.append(t)
                t = pp.tile([128, 2 * CQ], bf16, tag=f"wqk{i}", name=f"wqk{i}")
                qs[(2 * i + 1) % 3].dma_start(out=t[:], in_=wqk[128 * i:128 * (i + 1), :])
                wqk_sb.append(t)
            for i in range(KT):
                t = pp.tile([128, CQ], bf16, tag=f"wv{i}", name=f"wv{i}")
                qs[i % 3].dma_start(out=t[:], in_=wv[128 * i:128 * (i + 1), :])
                wv_sb.append(t)
            wo_sb = []
            for c in range(CQ // 128):
                t = pp.tile([128, DM], bf16, tag=f"wo{c}", name=f"wo{c}")
                qs[c % 3].dma_start(out=t[:], in_=wo[128 * c:128 * (c + 1), :])
                wo_sb.append(t)
            # [tri01 | identity]: tri01 for post-exp causal zeroing,
            # bf16 identity as the PE-transpose rhs
            mask_sb = pp.tile([128, 256], bf16, tag="mask", name="mask")
            nc.sync.dma_start(out=mask_sb[:], in_=mask[:, :])
            tri01 = mask_sb[:, 0:128]
            id_sb = mask_sb[:, 128:256]

            # persistent activation tensors
            qT = [pp.tile([128, S], bf16, tag=f"qT{p}", name=f"qT{p}") for p in range(NP)]
            kT = [pp.tile([128, S], bf16, tag=f"kT{p}", name=f"kT{p}") for p in range(NP)]
            v_rm = [pp.tile([128, NP * SLOT], bf16, tag=f"v{rt}", name=f"v{rt}") for rt in range(NRT)]
            oT = [pp.tile([128, S], bf16, tag=f"oT{p}", name=f"oT{p}") for p in range(NP)]

            if NO_ATTN:
                for p in range(NP):
                    nc.vector.memset(oT[p][:], 0.0)
            if ATTN_ONLY:
                for p in range(NP):
                    nc.vector.memset(qT[p][:], 0.01)
                    nc.vector.memset(kT[p][:], 0.01)
                for rt in range(NRT):
                    vw0 = v_rm[rt].rearrange("p (g s) -> p g s", s=SLOT)
                    nc.vector.memset(vw0[:, :, 0:64], 0.01)
                    nc.vector.memset(vw0[:, :, 65:129], 0.01)
            # static v_rm ones columns (denominator trick): col 64 even,
            # col 129 odd
            for rt in range(NRT):
                vw = v_rm[rt].rearrange("p (g s) -> p g s", s=SLOT)
                nc.vector.memset(vw[:, :, 64:65], 1.0)
                nc.vector.memset(vw[:, :, 129:130], 1.0)

            # ---------- filler unit machinery ----------
            # Units: [cost_ns, fn, emitted]. A tag maps to the (start, end)
            # queue range providing it; need(tag) emits just that range, out
            # of order if necessary. emit_units walks forward skipping
            # already-emitted entries.
            queue = []
            provided = {}    # tag -> (start, end) unit range
            ptr = {"pos": 0}

            def add_unit(cost, fn, head=False):
                queue.append([cost, fn, False, head])

            def mark(tag, start):
                provided[tag] = (start, len(queue))

            def emit_range(a, b):
                for i in range(a, b):
                    e = queue[i]
                    if not e[2]:
                        e[1]()
                        e[2] = True

            def close_open_group():
                # finish any half-emitted accumulation group at the pointer
                # so out-of-order pulls never open a second PSUM group
                pos = ptr["pos"]
                while pos < len(queue):
                    e = queue[pos]
                    if e[2]:
                        pos += 1
                        continue
                    if e[3]:
                        break  # next unit starts a fresh group: closed
                    e[1]()
                    e[2] = True
                    pos += 1
                ptr["pos"] = pos

            def need(tag):
                if tag in provided:
                    a, b = provided[tag]
                    if not (a <= ptr["pos"] and all(e[2] for e in queue[a:b])):
                        close_open_group()
                    emit_range(a, b)

            def emit_units(budget=None):
                # budget breaks only at group heads so a PSUM accumulation
                # group is never left half-emitted (pool-cycle hazard)
                pos = ptr["pos"]
                spent = 0
                while pos < len(queue):
                    e = queue[pos]
                    if e[2]:
                        pos += 1
                        continue
                    if budget is not None and spent + e[0] > budget:
                        break
                    e[1]()
                    e[2] = True
                    spent += e[0]
                    pos += 1
                ptr["pos"] = pos
                return spent

            # ---------- projection group builders (emit as units) ----------
            def qk_group(p, ct, rc):
                if ATTN_ONLY:
                    return
                dst = qT[p] if ct < NP else kT[p]
                g0 = len(queue)
                ps_box = {}
                for kt in range(KT):
                    def mm(kt=kt, ct=ct, rc=rc, ps_box=ps_box):
                        if kt == 0:
                            ps_box["ps"] = psmm.tile([128, 512], f32, tag="mm", name="mm")
                        nc.tensor.matmul(
                            ps_box["ps"][:],
                            lhsT=wqk_sb[kt][:, 128 * ct:128 * (ct + 1)],
                            rhs=xt_sb[kt][:, 512 * rc:512 * (rc + 1)],
                            start=(kt == 0),
                            stop=(kt == KT - 1),
                        )
                    add_unit(230, mm, head=(kt == 0))
                def evict(dst=dst, rc=rc, ps_box=ps_box):
                    nc.vector.tensor_copy(
                        dst[:, 512 * rc:512 * (rc + 1)], ps_box["ps"][:]
                    )
                add_unit(60, evict)
                mark(("qkg", p, ct, rc), g0)

            def qk_units(p):
                # Q then K column-tiles of pair p, all row chunks
                g0 = len(queue)
                for ct in (p, NP + p):
                    for rc in range(NRC):
                        qk_group(p, ct, rc)
                mark(("qk", p), g0)

            def v_units(rt):
                if ATTN_ONLY:
                    return
                g0 = len(queue)
                ps_box = {}
                for kt in range(KT):
                    def mm(kt=kt, rt=rt, ps_box=ps_box):
                        if kt == 0:
                            ps_box["ps"] = psmm.tile([128, 512], f32, tag="mm", name="mm")
                        nc.tensor.matmul(
                            ps_box["ps"][:],
                            lhsT=xt_sb[kt][:, 128 * rt:128 * (rt + 1)],
                            rhs=wv_sb[kt][:],
                            start=(kt == 0),
                            stop=(kt == KT - 1),
                        )
                    add_unit(230, mm, head=(kt == 0))
                def evict(rt=rt, ps_box=ps_box):
                    ps = ps_box["ps"]
                    psv = ps.rearrange("p (h c) -> p h c", c=64)
                    vw = v_rm[rt].rearrange("p (g s) -> p g s", s=SLOT)
                    nc.vector.tensor_copy(vw[:, :, 0:64], psv[:, 0::2, :])
                    nc.vector.tensor_copy(vw[:, :, 65:129], psv[:, 1::2, :])
                add_unit(120, evict)
                mark(("v", rt), g0)

            def oproj_units(rt):
                if ATTN_ONLY:
                    return
                st_box = {}
                for o2 in range(2):
                    ps_box = {}
                    for c in range(4):
                        def mm(c=c, o2=o2, rt=rt, ps_box=ps_box, st_box=st_box):
                            if c == 0 and o2 == 0:
                                st_box["sb"] = ostagep.tile(
                                    [128, 1024], bf16 if OUT_BF16 else f32,
                                    tag="ostage", name="ostage"
                                )
                            if c == 0:
                                ps_box["ps"] = psmm.tile([128, 512], f32, tag="mm", name="mm")
                            nc.tensor.matmul(
                                ps_box["ps"][:],
                                lhsT=oT[c][:, 128 * rt:128 * (rt + 1)],
                                rhs=wo_sb[c][:, 512 * o2:512 * (o2 + 1)],
                                start=(c == 0),
                                stop=(c == 3),
                            )
                        add_unit(230, mm, head=(c == 0))
                    def evict(o2=o2, ps_box=ps_box, st_box=st_box):
                        nc.vector.tensor_copy(
                            st_box["sb"][:, 512 * o2:512 * (o2 + 1)], ps_box["ps"][:]
                        )
                    add_unit(60, evict)
                def dma(rt=rt, st_box=st_box):
                    eng = nc.sync if rt % 2 == 0 else nc.gpsimd
                    eng.dma_start(
                        out=out[128 * rt:128 * (rt + 1), :], in_=st_box["sb"][:]
                    )
                add_unit(30, dma)

            # ---------- attention ----------
            def emit_sc_exp(p, st, kb):
                """Scores + mask + exp for one block; returns (pr, qi0)."""
                r = kb - 4 * st
                qi0 = 128 * r if r > 0 else 0
                sc = pssc.tile([128, 1024], f32, tag="sc", name="sc")
                for hh in range(2):
                    base, lo = 512 * hh, 64 * hh
                    nc.tensor.matmul(
                        sc[:, base + qi0:base + 512],
                        lhsT=kT[p][lo:lo + 64, 128 * kb:128 * (kb + 1)],
                        rhs=qT[p][lo:lo + 64, 512 * st + qi0:512 * (st + 1)],
                        start=True,
                        stop=True,
                        tile_position=(lo, 0),
                    )
                pr = probsp.tile([128, 1024], bf16, tag="pr", name="pr")
                if qi0 == 0:
                    nc.scalar.activation(pr[:], sc[:], AF.Exp)
                else:
                    pr_v = pr.rearrange("p (h q) -> p h q", h=2)
                    sc_v = sc.rearrange("p (h q) -> p h q", h=2)
                    nc.scalar.activation(
                        pr_v[:, :, qi0:512], sc_v[:, :, qi0:512], AF.Exp
                    )
                if r >= 0:
                    # causal zeroing after exp (keeps DVE off the Act path):
                    # strip cols [qi0, qi0+128) follow the same strictly-
                    # lower-tri pattern (p > jj) for every diagonal block
                    for hh in range(2):
                        base = 512 * hh
                        nc.gpsimd.tensor_mul(
                            pr[:, base + qi0:base + qi0 + 128],
                            pr[:, base + qi0:base + qi0 + 128],
                            tri01,
                        )
                return pr, qi0

            def emit_pv(p, out_ps, st, kb, pr):
                """q-major PV: chunk j (128 q) x head h -> [128q, 65] (vals+den).

                out_ps = (tileA, tileB), each [128, 260] holding two chunks'
                slots of 65 cols. Chunk j accumulates kb = 0 .. 4*st+j.
                """
                need(("v", kb))
                r = kb - 4 * st
                vw = v_rm[kb].rearrange("p (g s) -> p g s", s=SLOT)
                # PSUM accumulation groups are BANK-granular (start zeroes the
                # whole 2KB zero-region): exactly one start (first matmul at
                # kb==0) and one stop (the bank's last matmul: odd chunk's
                # diagonal block, odd head) per tile per supertile.
                for j in range(max(r, 0), 4):
                    t = out_ps[j // 2]
                    for h in range(2):
                        col0 = 130 * (j % 2) + 65 * h
                        nc.tensor.matmul(
                            t[:, col0:col0 + 65],
                            lhsT=pr[:, 512 * h + 128 * j:512 * h + 128 * (j + 1)],
                            rhs=vw[:, p, 65 * h:65 * h + 65],
                            start=(kb == 0 and h == 0 and j % 2 == 0),
                            stop=(j % 2 == 1 and kb == 4 * st + j and h == 1),
                        )

            def normalize_half(p, st, out_ps, half, box):
                """recip + per-partition scale of one half-supertile (2 chunks).

                box carries the shared bf16 stage tile across halves; the
                second half returns the deferred post (PE transposes + oT
                eviction)."""
                t = out_ps[half]
                v4 = t.rearrange("p (s c) -> p s c", c=65)
                rc = normp.tile([128, 4], f32, tag="rc", name="rc")
                nc.vector.reciprocal(rc[:, 0:4], v4[:, :, 64])
                if half == 0:
                    box["stage"] = normp.tile([128, 512], bf16, tag="stage",
                                              name="stage")
                stage = box["stage"]
                for jl in range(2):
                    j = 2 * half + jl
                    for h in range(2):
                        s = 2 * jl + h
                        nc.vector.tensor_scalar_mul(
                            stage[:, 128 * j + 64 * h:128 * j + 64 * h + 64],
                            v4[:, s, 0:64],
                            rc[:, s:s + 1],
                        )
                if half == 0:
                    return None

                def post():
                    tp_ps = psmm.tile([128, 512], f32, tag="mm", name="mm")
                    tpb = tp_ps[:].bitcast(bf16)
                    # one bank-wide group: start zeroes the bank, chunks land
                    # in disjoint col ranges, single stop; bf16 transposes
                    # (1 cyc/row) into a bitcast view of the f32 mm tile
                    for j in range(4):
                        nc.tensor.matmul(
                            tpb[:, 128 * j:128 * (j + 1)],
                            lhsT=stage[:, 128 * j:128 * (j + 1)],
                            rhs=id_sb[:],
                            is_transpose=True,
                            start=(j == 0),
                            stop=(j == 3),
                        )
                    nc.vector.tensor_copy(
                        oT[p][:, 512 * st:512 * (st + 1)], tpb[:, 0:512]
                    )

                return post

            # ---------- schedule ----------
            ACT_OVH = tune.get("act_ovh", 295)   # per-activation fixed ns
            ACT_RATE = tune.get("act_rate", 1.2)  # exp cols per ns
            BLK_PAD = tune.get("blk_pad", 0)      # extra per-block HW latency

            for _arep in range(rep_all):
             pending_post = []
             provided[("repstart", _arep)] = (0, len(queue))
             first_flush = [True]

             if JIT_QK:
                 if _arep == 0:
                     for rc in range(NRC):
                         qk_group(0, 0, rc)
                         qk_group(0, NP, rc)
                         for rt in range(4 * rc, 4 * rc + 4):
                             v_units(rt)
                 else:
                     # qk(0) was queued during the previous rep's last pair
                     for rt in range(NRT):
                         v_units(rt)
                 for _qrep in range(max(rep_qkv, 1) - 1):
                     qk_units(0)
                 for p in range(1, NP):
                     for _qrep in range(max(rep_qkv, 1)):
                         qk_units(p)
                 # pre-phase: only q/k row-chunk 0 of pair 0 + v tile 0
                 need(("qkg", 0, 0, 0))
                 need(("qkg", 0, NP, 0))
                 need(("v", 0))
             else:
                 for _qrep in range(max(rep_qkv, 1)):
                     qk_units(0)
                 for rt in range(NRT):
                     v_units(rt)
                 for p in range(1, NP):
                     for _qrep in range(max(rep_qkv, 1)):
                         qk_units(p)
                 # pre-phase: q/k of pair 0 + first v tiles
                 need(("qk", 0))
                 need(("v", 3))

             if NO_ATTN:
                 emit_units()
                 for rt in range(NRT):
                     oproj_units(rt)
                 emit_units()
                 continue

             for p in range(NP):
                 if JIT_QK:
                     need(("qkg", p, p, 0))
                     need(("qkg", p, NP + p, 0))
                     if p == NP - 1 and _arep + 1 < rep_all:
                         # queue next rep's q/k of pair 0 as fillers for the
                         # last pair's slack (qT[0]/kT[0] are long dead here)
                         for rc in range(NRC):
                             qk_group(0, 0, rc)
                             qk_group(0, NP, rc)
                 else:
                     need(("qk", p))
                 def finish_st(prev):
                     # prev = (out_ps, st, kb, pr, box)
                     out_ps, pst, pkb, ppr, pbox = prev
                     emit_pv(p, out_ps, pst, pkb, ppr)
                     post = normalize_half(p, pst, out_ps, 1, pbox)
                     if p == NP - 1 and rep_attn == 1:
                         def put(pst=pst, post=post):
                             post()
                             for rt in range(4 * pst, 4 * pst + 4):
                                 oproj_units(rt)
                         pending_post.append(put)
                     else:
                         pending_post.append(post)

                 def process(ent):
                     if ent[2] == 4 * ent[1] + 3:
                         finish_st(ent)    # last block of its supertile
                     else:
                         emit_pv(p, ent[0], ent[1], ent[2], ent[3])
                         if ent[2] == 4 * ent[1] + 1:
                             # first half-supertile complete: free tileA
                             normalize_half(p, ent[1], ent[0], 0, ent[4])

                 blocks = [
                     (st, kb)
                     for st in range(NST)
                     for _ in range(rep_attn)
                     for kb in range(4 * st + 4)
                 ]
                 out_ps = None
                 norm_box = None
                 pend = deque()   # PV lags exp by PV_LAG blocks: accumulation
                 deficit = 0      # is commutative, and an aged pr is sem-ready
                 blk = 0          # so the in-order PE never stalls on Act
                 for st, kb in blocks:
                     if kb == 0:
                         if JIT_QK:
                             need(("qkg", p, p, st))
                         out_ps = [
                             psout.tile([128, 260], f32, tag="o", name="o")
                             for _ in range(2)
                         ]
                         norm_box = {}
                     if JIT_QK:
                         need(("qkg", p, NP + p, kb // 4))
                     pr, qi0 = emit_sc_exp(p, st, kb)
                     ncols = 2 * (512 - qi0)
                     act_ns = ncols / ACT_RATE + ACT_OVH + BLK_PAD
                     r_ = kb - 4 * st
                     # sc matmuls + q-major PV (2 heads x (4-max(r,0)) chunks
                     # of 65 cols) + per-block fixed overhead
                     pe_ns = ncols / 2.4 + 130 * (4 - max(r_, 0)) / 2.4 + 60
                     deficit += act_ns - pe_ns
                     if pending_post and blk >= DEFER_KB:
                         if first_flush[0]:
                             # all leftover units from the previous rep
                             # must precede this rep's first oT write
                             need(("repstart", _arep))
                             first_flush[0] = False
                         for fn in pending_post:
                             fn()
                         pending_post.clear()
                         deficit -= 450
                     if deficit > 0:
                         deficit -= emit_units(budget=deficit)
                     while len(pend) >= max(PV_LAG, 1):
                         process(pend.popleft())
                     pend.append((out_ps, st, kb, pr, norm_box))
                     blk += 1
                 while pend:
                     process(pend.popleft())

             for fn in pending_post:
                 fn()
             pending_post.clear()
             if p == NP - 1 and rep_attn != 1:
                 for rt in range(NRT):
                     oproj_units(rt)
             for _orep in range(rep_oproj - 1):
                 for rt in range(NRT):
                     oproj_units(rt)
             if _arep + 1 >= rep_all:
                 emit_units()  # final drain (tail out-proj)
    nc.finalize()
    return nc


def get_program():
    if "nc" not in _PROG_CACHE:
        _PROG_CACHE["nc"] = build_program()
    return _PROG_CACHE["nc"]


def make_in_maps(x, w_qkv, w_out):
    import ml_dtypes

    bf = ml_dtypes.bfloat16
    x = np.asarray(x, dtype=np.float32)
    w_qkv = np.asarray(w_qkv, dtype=np.float32)
    w_out = np.asarray(w_out, dtype=np.float32)
    scale = float(D) ** -0.5
    # post-exp causal zeroing: strictly-lower-tri 0/1 (p > jj masked); the
    # same [128,128] pattern applies to every diagonal block's strip.
    # Identity appended as the PE-transpose rhs.
    p_idx = np.arange(128)[:, None]
    j_idx = np.arange(128)[None, :]
    tri = np.where(p_idx > j_idx, 0.0, 1.0)
    mask = np.concatenate([tri, np.eye(128)], axis=1).astype(bf)
    in_maps = []
    for c in range(NCORES):
        b, hh = c // 2, c % 2
        q0 = CQ * hh
        wq = (w_qkv[:, q0:q0 + CQ] * scale).astype(bf)
        wk = w_qkv[:, DM + q0:DM + q0 + CQ].astype(bf)
        in_maps.append(
            {
                "xt": np.ascontiguousarray(x[b].T).astype(bf),
                "wqk": np.concatenate([wq, wk], axis=1),
                "wv": w_qkv[:, 2 * DM + q0:2 * DM + q0 + CQ].astype(bf),
                "wo": w_out[q0:q0 + CQ, :].astype(bf),
                "mask": mask,
            }
        )
    return in_maps


def gather(results):
    outs = [np.asarray(results[c]["out"], dtype=np.float32) for c in range(NCORES)]
    return np.stack([outs[2 * b] + outs[2 * b + 1] for b in range(B)], axis=0)


def kernel(x, w_qkv, w_out):
    from concourse.bass_utils import run_bass_kernel_spmd

    nc = get_program()
    in_maps = make_in_maps(x, w_qkv, w_out)
    res = run_bass_kernel_spmd(nc, in_maps, list(range(NCORES)))
    return gather(res.results)

